# revision 10
# baseline (speedup 1.0000x reference)
"""Trainium2 Bass kernel for nn_BreakthroughSNN (predictive-coding SNN LM), v2.

Improvements over v1 baseline:
  - TD j=2 gen matmul reused as xgen[2] (they are identical: td_in = states[2]).
  - Gen-bias matmuls removed: gen_b folded in T-layout via DVE adds.
  - LIF updates batched across layers ([128,192] ops): BU gen-LIF and TD
    inf-LIF each collapse 9 DVE ops -> 3.
  - LN stats chain shortened: work in raw-sum space
    (istd = D/sqrt(s2*D - s1^2 + eps*D^2)), removing 2 ops per LN.
  - Matmul dtype selectable (fp32 / fp32r): fp32 PE matmuls cost 4 cyc/row;
    fp32r is 1 cyc/row at N>=512.
  - Vocab projection in bf16 (td spikes are exactly representable; out_W
    rounding ~0.4% rel) and interleaved into the step loop (fills PE idle
    time, keeps HAM warm, overlaps the output DMA).
  - LN2 per-batch stats read directly from PSUM (skips a copy).
"""

import os
import sys

sys.path.insert(0, "/opt/trn_rl_repo")

import numpy as np
import ml_dtypes

import concourse.bass as bass
import concourse.bacc as bacc
from concourse import mybir
from concourse.bass_utils import run_bass_kernel_spmd
from concourse.tile import TileContext

F32 = mybir.dt.float32
F32R = mybir.dt.float32r
BF16 = mybir.dt.bfloat16
Alu = mybir.AluOpType
Act = mybir.ActivationFunctionType

B, S, V, D, L = 16, 128, 32000, 512, 3
NCORES = 8
VS = V // NCORES  # 4000 vocab rows per core
DECAY = float(np.exp(-1.0 / 2.0))
THR = 1.0
EPS = 1e-5
C = D // 128  # 4 d-chunks
W64 = C * B  # 64: free width of a T-layout tile

# matmul input dtype for the recurrence matmuls (fp32r = 4x faster on PE,
# slightly reduced precision). Projection is always bf16.
MM_F32R = True


def _wT_kmajor(W, dt=np.float32):
    """W: (D_out, D_in) -> SBUF layout [128, C*D_out], k-chunk-major.

    slice [:, k*D_out:(k+1)*D_out] is W.T[k*128:(k+1)*128, :]."""
    Dout, Din = W.shape
    return (
        np.ascontiguousarray(W.T)
        .reshape(Din // 128, 128, Dout)
        .transpose(1, 0, 2)
        .reshape(128, (Din // 128) * Dout)
        .astype(dt)
    )


def _vec_T(v):
    """v: (D,) -> T-layout tile [128, 64] (b-independent broadcast)."""
    t = v.reshape(C, 128).T  # [128, C]
    return np.ascontiguousarray(
        np.broadcast_to(t[:, :, None], (128, C, B)).reshape(128, W64)
    ).astype(np.float32)


def _tok_T(tok):
    """tok: (B,S,D) -> c-major [128, C*S*16]: [p, c*S*16 + t*16 + b]."""
    S_ = tok.shape[1]
    a = tok.transpose(2, 1, 0)  # [D, S, B]
    a = a.reshape(C, 128, S_, B).transpose(1, 0, 2, 3)  # [128, C, S, B]
    return np.ascontiguousarray(a.reshape(128, C * S_ * B)).astype(np.float32)


def _build(nS):
    """Build the Bass program for nS recurrence steps."""
    nc = bacc.Bacc(None, target_bir_lowering=False)

    # ---- DRAM parameters ----
    d_tok = nc.declare_dram_parameter("tok_t", [128, nS * W64], BF16, isOutput=False)
    d_encw = nc.declare_dram_parameter("encw_t", [128, C * D], BF16, isOutput=False)
    d_genw = nc.declare_dram_parameter("genw_t", [L, 128, C * D], BF16, isOutput=False)
    d_infw = nc.declare_dram_parameter("infw_t", [L, 128, C * D], BF16, isOutput=False)
    d_outw = nc.declare_dram_parameter("outw_t", [128, C * VS], BF16, isOutput=False)
    # small constants, stacked T-layout tiles along free dim [128, 15*64]:
    # 0..2 genb_T, 3..5 B2_T, 6..8 G2_T, 9..11 nsg_T, 12..14 nsb_T
    d_ctile = nc.declare_dram_parameter("ctiles", [128, 15 * W64], BF16, isOutput=False)
    d_encb = nc.declare_dram_parameter("encb_c", [128, C], F32, isOutput=False)
    d_ones = nc.declare_dram_parameter("ones_in", [128, 128], BF16, isOutput=False)
    d_logits = nc.declare_dram_parameter("logits", [nS * B, VS], BF16, isOutput=True)

    # projection m-blocks: each covers 128 (t,b) pairs = 8 steps
    n_mb = (nS * B) // 128 if nS * B >= 128 else 1
    t_mb = nS // n_mb  # steps per M-block
    NB = VS // 500  # 8 chunks of 500 vocab cols

    with TileContext(nc) as tc:
        with (
            tc.tile_pool(name="const", bufs=1) as cpool,
            tc.tile_pool(name="state", bufs=1) as spool,
            tc.tile_pool(name="work", bufs=4) as wpool,
            tc.tile_pool(name="stat", bufs=6) as stpool,
            tc.tile_pool(name="fin", bufs=4) as fpool,
            tc.tile_pool(name="owq", bufs=2) as owq,
            tc.tile_pool(name="psB", bufs=2, space="PSUM") as psB,
            tc.tile_pool(name="psS", bufs=2, space="PSUM") as psS,
            tc.tile_pool(name="psT", bufs=1, space="PSUM") as psT,
            tc.tile_pool(name="psP", bufs=1, space="PSUM") as psP,
        ):
            # ---- load constants / weights into SBUF ----
            encw = cpool.tile([128, C * D], BF16, name="encw")
            nc.sync.dma_start(out=encw, in_=d_encw[:, :])
            genw = [cpool.tile([128, C * D], BF16, tag=f"genw{j}", name=f"genw{j}") for j in range(L)]
            infw = [cpool.tile([128, C * D], BF16, tag=f"infw{j}", name=f"infw{j}") for j in range(L)]
            for j in range(L):
                nc.sync.dma_start(out=genw[j], in_=d_genw[j])
                nc.sync.dma_start(out=infw[j], in_=d_infw[j])
            ct = cpool.tile([128, 15 * W64], BF16, name="ct")
            nc.sync.dma_start(out=ct, in_=d_ctile[:, :])
            _cs = lambda i: ct[:, i * W64 : (i + 1) * W64]
            genbT = [_cs(j) for j in range(3)]
            B2T = [_cs(3 + j) for j in range(3)]
            B2Tall = ct[:, 3 * W64 : 6 * W64]
            G2T = [_cs(6 + j) for j in range(3)]
            nsgT = [_cs(9 + j) for j in range(3)]
            nsbT = [_cs(12 + j) for j in range(3)]

            encb = cpool.tile([128, C], F32, name="encb")
            nc.sync.dma_start(out=encb, in_=d_encb[:, :])

            onesin = cpool.tile([128, 128], BF16, name="onesin")
            nc.sync.dma_start(out=onesin, in_=d_ones[:, :])
            ones_col = onesin[:, 0:1]  # [128,1] ones (lhsT for stat-mm)
            ones_row = onesin[0:1, :]  # [1,128] ones (lhsT for bcast-mm)

            zeros3 = cpool.tile([128, 3 * W64], BF16, name="zeros3")
            nc.vector.memset(zeros3, 0.0)
            zeros = zeros3[:, 0:W64]
            onethr = cpool.tile([128, W64], BF16, name="onethr")
            nc.vector.memset(onethr, THR)
            # 1 - genbT, for the TD-chain spike-threshold trick
            genbT1c = cpool.tile([128, 3 * W64], BF16, name="genbT1c")
            nc.vector.tensor_scalar(
                out=genbT1c, in0=ct[:, 0 : 3 * W64], scalar1=-1.0, scalar2=THR,
                op0=Alu.mult, op1=Alu.add,
            )
            epst = cpool.tile([128, 1], F32, name="epst")
            nc.vector.memset(epst, EPS * D * D)

            tok = cpool.tile([128, nS * W64], BF16, name="tok")
            nc.sync.dma_start(out=tok, in_=d_tok[:, :])
            xenc = cpool.tile([128, nS * W64], BF16, name="xenc")
            # bf16 td history, c-major [p, c*(nS*16) + t*16 + b]
            tdh = cpool.tile([128, nS * W64], BF16, name="tdh")

            # ---- persistent state tiles ----
            mem_enc = spool.tile([128, W64], BF16, tag="mem_enc", name="mem_enc")
            mem_gen = spool.tile([128, 3 * W64], BF16, tag="mgen", name="mgen")
            mem_inf = spool.tile([128, 3 * W64], BF16, tag="minf", name="minf")
            states = [spool.tile([128, W64], BF16, tag=f"st{j}", name=f"stt{j}") for j in range(L)]
            xgen = spool.tile([128, 3 * W64], BF16, tag="xg", name="xg")
            # thr_x = 1 - DECAY*mem_x, maintained at the end of each step
            thre = spool.tile([128, W64], BF16, tag="thre", name="thre")
            thrg = spool.tile([128, 3 * W64], BF16, tag="thrg", name="thrg")

            def jsl(tile, j):
                return tile[:, j * W64 : (j + 1) * W64]

            nc.vector.memset(mem_enc, 0.0)
            nc.vector.memset(mem_gen, 0.0)
            nc.vector.memset(mem_inf, 0.0)
            nc.vector.memset(thre, THR)
            nc.vector.memset(thrg, THR)
            for j in range(L):
                nc.vector.memset(states[j], 0.0)
                # states0 = 0 -> x_gen(t=0) = gen_b
                nc.vector.tensor_copy(jsl(xgen, j), genbT[j])

            # identity for PE transpose (bf16 to match the staging tiles)
            ident = cpool.tile([128, 128], BF16, name="ident")
            from concourse.masks import make_identity

            make_identity(nc, ident)

            # PE pre-touch of encw so the first real matmul carries only one
            # DMA-queue wait.
            ptch = psS.tile([16, 1], BF16, tag="psmall", name="ptch")
            nc.tensor.transpose(ptch, encw[0:1, 0:16], onesin[0:1, 0:1])

            # ---- prologue: x_enc = tok @ enc_W.T + enc_b, all steps ----
            TB = nS * B  # cols per c-chunk in c-major tok layout
            xenc4 = xenc.rearrange("p (t c b) -> p c t b", c=C, b=B)
            nblk = nS * W64 // 512 if nS * W64 >= 512 else 1
            tblk = nS // nblk  # steps per 512-col block
            for tc_i in range(nblk):
                for m in range(C):
                    pE = psB.tile([128, 512], F32, tag="pbig", name="pE")
                    fd = tblk * B
                    for k in range(C):
                        nc.tensor.matmul(
                            pE[:, :fd],
                            (encw[:, k * D + m * 128 : k * D + m * 128 + 128]),
                            (tok[:, k * TB + tc_i * fd : k * TB + (tc_i + 1) * fd]),
                            start=(k == 0),
                            stop=(k == C - 1),
                        )
                    nc.vector.tensor_scalar(
                        out=xenc4[:, m, tc_i * tblk : (tc_i + 1) * tblk, :],
                        in0=pE[:, :fd].rearrange("p (t b) -> p t b", b=B),
                        scalar1=encb[:, m : m + 1],
                        scalar2=None,
                        op0=Alu.add,
                    )

            # ---- helper closures ----
            def ln_stats(vtile, tag):
                """vtile: [128,128] with v in cols 0:64 and v^2 in 64:128.

                Stats via ones[128,128] stationary matmul: every output
                partition receives the same per-(g,b) sums; the c-fold is
                done by PSUM accumulation over 4 chunked matmuls. Result is
                broadcast across partitions for free (no bcast-mm, no
                free-dim reduce). Returns bsrc SBUF [128,32]:
                0:16 istd, 16:32 mu*istd."""
                pst = psS.tile([128, 32], F32, tag="pstat", name="pst")
                v4 = vtile.rearrange("q (g c b) -> q g c b", g=2, c=C)
                p3 = pst.rearrange("p (g b) -> p g b", g=2)
                for c in range(C):
                    nc.tensor.matmul(
                        p3, onesin, v4[:, :, c, :],
                        start=(c == 0), stop=(c == C - 1),
                    )
                st = stpool.tile([128, 32], F32, tag="st_s", name="st")
                nc.vector.tensor_copy(st, pst)
                sq = stpool.tile([128, 16], F32, tag="sq_s", name="sq")
                nc.vector.tensor_mul(sq, st[:, 0:16], st[:, 0:16])
                # v = s2*D - s1^2  ( = D^2 * var )
                v = stpool.tile([128, 16], F32, tag="v_s", name="v")
                nc.vector.scalar_tensor_tensor(
                    out=v,
                    in0=st[:, 16:32],
                    scalar=float(D),
                    in1=sq,
                    op0=Alu.mult,
                    op1=Alu.subtract,
                )
                # sd = sqrt(v + eps*D^2) = D * sqrt(var + eps)
                sd = stpool.tile([128, 16], F32, tag="sd_s", name="sd")
                nc.scalar.activation(out=sd, in_=v, func=Act.Sqrt, bias=epst)
                rc = stpool.tile([128, 16], F32, tag="rc_s", name="rc")
                nc.vector.reciprocal_approx_fast(rc, sd)  # 1/(D*sqrt(var+eps))
                bsrc = stpool.tile([128, 32], BF16, tag="bs_s", name="bs")
                nc.vector.tensor_scalar(
                    out=bsrc[:, 0:16], in0=rc, scalar1=float(D), scalar2=None,
                    op0=Alu.mult,
                )  # istd
                nc.vector.tensor_mul(bsrc[:, 16:32], st[:, 0:16], rc)  # mu*istd
                return bsrc

            def bc(pbc, lo):
                """[128,16] slice of pbc broadcast to [128,4,16]."""
                return pbc[:, None, lo : lo + 16].broadcast_to([128, C, 16])

            def as3(t):
                return t.rearrange("p (c b) -> p c b", c=C)

            def mm_TN(psum, lhsT64, w_sb):
                """psum [16,512] = lhsT64.T @ W.T (4 k-chunk accumulation)."""
                for k in range(C):
                    nc.tensor.matmul(
                        psum,
                        (lhsT64[:, k * B : (k + 1) * B]),
                        (w_sb[:, k * D : (k + 1) * D]),
                        start=(k == 0),
                        stop=(k == C - 1),
                    )

            def transpose_NT2(xN, tag):
                pT = psS.tile([128, W64], BF16, tag="psmall", name="pT")
                for c in range(C):
                    nc.tensor.transpose(
                        pT[:, c * B : (c + 1) * B],
                        xN[:, c * 128 : (c + 1) * 128],
                        ident[0:16, 0:16],
                    )
                return pT

            tdh4 = tdh.rearrange("p (c t b) -> p c t b", c=C, b=B)

            # projection schedule: (m-block, vocab-chunk) pairs per step.
            # m-block mb is complete after step (mb+1)*t_mb - 1; spread its NB
            # chunks over the following steps, one per step; whatever remains
            # after the last step runs in the epilogue (scheduled on t=nS-1).
            proj_sched = {}
            for mb in range(n_mb):
                for nb in range(NB):
                    t_issue = (mb + 1) * t_mb + nb
                    proj_sched.setdefault(min(t_issue, nS - 1), []).append((mb, nb))

            # ---- main recurrence ----
            for t in range(nS):
                xenc_t = xenc[:, t * W64 : (t + 1) * W64]
                # spikes via precomputed thresholds (single hop from xgen)
                bu0 = wpool.tile([128, W64], BF16, tag="bu0", name="bu0")
                nc.vector.tensor_tensor(out=bu0, in0=xenc_t, in1=thre, op=Alu.is_ge)
                predall = wpool.tile([128, 3 * W64], BF16, tag="predall", name="predall")
                nc.vector.tensor_tensor(out=predall, in0=xgen, in1=thrg, op=Alu.is_ge)
                # membrane updates + resets (off the critical chain)
                nc.vector.scalar_tensor_tensor(
                    out=mem_enc, in0=mem_enc, scalar=DECAY, in1=xenc_t,
                    op0=Alu.mult, op1=Alu.add,
                )
                nc.vector.copy_predicated(mem_enc, bu0.bitcast(mybir.dt.int16), zeros)
                nc.vector.scalar_tensor_tensor(
                    out=mem_gen, in0=mem_gen, scalar=DECAY, in1=xgen,
                    op0=Alu.mult, op1=Alu.add,
                )
                nc.vector.copy_predicated(
                    mem_gen, predall.bitcast(mybir.dt.int16), zeros3
                )
                # next-step encoder threshold (xenc is static, so the whole
                # encoder chain is off the critical path)
                nc.vector.scalar_tensor_tensor(
                    out=thre, in0=mem_enc, scalar=-DECAY, in1=onethr,
                    op0=Alu.mult, op1=Alu.add,
                )
                # TD-chain thresholds: thr2_j = 1 - DECAY*mem_gen_j - genbT_j
                thr2 = wpool.tile([128, 3 * W64], BF16, tag="thr2", name="thr2")
                nc.vector.scalar_tensor_tensor(
                    out=thr2, in0=mem_gen, scalar=-DECAY, in1=genbT1c,
                    op0=Alu.mult, op1=Alu.add,
                )
                # bp_j = ns_b[j-1] - pred_j (fused next-layer residual base)
                bp = [None] * L
                for j in (1, 2):
                    bp[j] = wpool.tile([128, W64], BF16, tag=f"bp{j}", name=f"bp{j}")
                    nc.vector.tensor_sub(bp[j], nsbT[j - 1], jsl(predall, j))

                # ---- bottom-up ----
                r2_next = None
                for j in range(L):
                    # r = relu(bu - pred); for j>0 the subtraction was fused
                    # into the previous layer's LN apply (r2_next)
                    if j == 0:
                        r2 = wpool.tile([128, 2 * W64], BF16, tag="r2", name="r2")
                        r = r2[:, 0:W64]
                        nc.vector.tensor_sub(r, bu0, jsl(predall, 0))
                    else:
                        r2 = r2_next
                        r = r2[:, 0:W64]
                    nc.vector.tensor_scalar(
                        out=r, in0=r, scalar1=0.0, scalar2=None, op0=Alu.max
                    )
                    # LN1 stats concurrent with matmul
                    nc.vector.tensor_mul(r2[:, W64 : 2 * W64], r, r)
                    bc1 = ln_stats(r2, f"ln1_{j}")

                    # inf matmul on r (g folded into weights)
                    pM = psB.tile([16, 512], F32, tag="pbig", name="pM")
                    mm_TN(pM, r, infw[j])
                    xN = wpool.tile([16, 512], BF16, tag="nbuf", name="xN")
                    nc.scalar.copy(xN, pM)
                    pT = transpose_NT2(xN, "inf")

                    # mem_inf[j] = mem_inf[j]*dec + istd*rT + B2 - mu*istd*G2
                    base = wpool.tile([128, W64], BF16, tag="base", name="base")
                    nc.vector.scalar_tensor_tensor(
                        out=base,
                        in0=jsl(mem_inf, j),
                        scalar=DECAY,
                        in1=B2T[j],
                        op0=Alu.mult,
                        op1=Alu.add,
                    )
                    gsc = wpool.tile([128, W64], BF16, tag="gsc", name="gsc")
                    nc.vector.tensor_mul(as3(gsc), as3(G2T[j]), bc(bc1, 16))
                    nc.vector.tensor_sub(base, base, gsc)
                    xsc = wpool.tile([128, W64], BF16, tag="xsc", name="xsc")
                    nc.vector.tensor_mul(as3(xsc), as3(pT), bc(bc1, 0))
                    nc.vector.tensor_add(jsl(mem_inf, j), xsc, base)
                    # wv = states + spike(mem_inf), spike fused into the add
                    w2 = wpool.tile([128, 2 * W64], BF16, tag="w2", name="w2")
                    wv = w2[:, 0:W64]
                    nc.vector.scalar_tensor_tensor(
                        out=wv, in0=jsl(mem_inf, j), scalar=THR, in1=states[j],
                        op0=Alu.is_ge, op1=Alu.add,
                    )
                    # reset (off-chain): mem *= (mem < THR)
                    mli = wpool.tile([128, W64], BF16, tag="mli", name="mli")
                    nc.vector.tensor_scalar(
                        out=mli, in0=jsl(mem_inf, j), scalar1=THR, scalar2=None,
                        op0=Alu.is_lt,
                    )
                    nc.vector.tensor_mul(jsl(mem_inf, j), jsl(mem_inf, j), mli)

                    # state' = LN_ns(wv)
                    nc.vector.tensor_mul(w2[:, W64 : 2 * W64], wv, wv)
                    pbc2 = ln_stats(w2, f"ln2_{j}")
                    t1 = wpool.tile([128, W64], BF16, tag="t1", name="t1")
                    nc.vector.tensor_mul(as3(t1), as3(wv), bc(pbc2, 0))
                    nc.vector.tensor_tensor(
                        out=as3(t1), in0=as3(t1), in1=bc(pbc2, 16), op=Alu.subtract
                    )
                    nc.vector.tensor_mul(t1, t1, nsgT[j])
                    nc.vector.tensor_add(states[j], t1, nsbT[j])
                    if j < 2:
                        # fused next-layer residual: r_{j+1} pre-relu
                        # = states[j] - pred_{j+1} = t1 + (nsbT - pred_{j+1})
                        r2_next = wpool.tile([128, 2 * W64], BF16, tag="r2", name="r2")
                        nc.vector.tensor_add(r2_next[:, 0:W64], t1, bp[j + 1])

                # ---- top-down ----
                td_in = states[L - 1]
                for j in reversed(range(L)):
                    # gen matmul for TD chain; for j=2 it doubles as xgen[2]
                    pTD = psT.tile([16, 512], F32, tag="ptd", name="pTD")
                    mm_TN(pTD, td_in, genw[j])
                    tdN = wpool.tile([16, 512], BF16, tag="nbuf", name="tdN")
                    nc.scalar.copy(tdN, pTD)
                    pTDt = transpose_NT2(tdN, "td")

                    # spike via threshold trick: one hop from the transpose
                    p_j = wpool.tile([128, W64], BF16, tag=f"p_td{j}", name="p_j")
                    nc.vector.tensor_tensor(
                        out=p_j, in0=pTDt, in1=jsl(thr2, j), op=Alu.is_ge
                    )
                    # membrane update + reset (off the critical chain)
                    tmpg = wpool.tile([128, W64], BF16, tag="tmpg", name="tmpg")
                    nc.vector.scalar_tensor_tensor(
                        out=tmpg,
                        in0=jsl(mem_gen, j),
                        scalar=DECAY,
                        in1=genbT[j],
                        op0=Alu.mult,
                        op1=Alu.add,
                    )
                    nc.vector.tensor_add(jsl(mem_gen, j), tmpg, pTDt)
                    if j == 2:
                        # xgen[2] = states[2]@genW2 + gen_b2 (same matmul)
                        nc.vector.tensor_add(jsl(xgen, 2), pTDt, genbT[2])
                    if j > 0:
                        td_in = p_j
                    else:
                        # bf16 td history write (c-major), for the projection
                        nc.scalar.copy(tdh4[:, :, t, :], as3(p_j))

                # xgen for next step, layers 0/1 (off critical path)
                for j in range(2):
                    pXG = psB.tile([16, 512], F32, tag="pbig", name="pXG")
                    mm_TN(pXG, states[j], genw[j])
                    xgN = wpool.tile([16, 512], BF16, tag="nbuf", name="xgN")
                    nc.scalar.copy(xgN, pXG)
                    pXGt = transpose_NT2(xgN, "xg")
                    nc.vector.tensor_add(jsl(xgen, j), pXGt, genbT[j])

                # batched TD gen reset: mem_gen *= (mem_gen < THR)
                mltg = wpool.tile([128, 3 * W64], BF16, tag="mltg", name="mltg")
                nc.vector.tensor_scalar(
                    out=mltg, in0=mem_gen, scalar1=THR, scalar2=None, op0=Alu.is_lt
                )
                nc.vector.tensor_mul(mem_gen, mem_gen, mltg)
                # batched TD inf-LIF (input is the constant B2; spikes unused)
                nc.vector.scalar_tensor_tensor(
                    out=mem_inf,
                    in0=mem_inf,
                    scalar=DECAY,
                    in1=B2Tall,
                    op0=Alu.mult,
                    op1=Alu.add,
                )
                si = wpool.tile([128, 3 * W64], BF16, tag="si", name="si")
                nc.vector.tensor_scalar(
                    out=si, in0=mem_inf, scalar1=THR, scalar2=None, op0=Alu.is_ge
                )
                nc.vector.copy_predicated(mem_inf, si.bitcast(mybir.dt.int16), zeros3)
                # next-step gen spike thresholds
                nc.vector.tensor_scalar(
                    out=thrg, in0=mem_gen, scalar1=-DECAY, scalar2=THR,
                    op0=Alu.mult, op1=Alu.add,
                )

                # ---- interleaved projection: one vocab chunk per step ----
                # (keeps the PE HAM-warm and overlaps the logits DMA)
                for mb, nb in proj_sched.get(t, ()):
                    fd = t_mb * B  # 128
                    outwq = owq.tile([128, C * 500], BF16, tag="outwq", name="outwq")
                    nc.sync.dma_start(
                        out=outwq,
                        in_=d_outw[:, nb * C * 500 : (nb + 1) * C * 500],
                    )
                    pf = psP.tile([128, 512], F32, tag="pproj", name="pf")
                    for k in range(C):
                        nc.tensor.matmul(
                            pf[:fd, 0:500],
                            tdh[:, k * TB + mb * fd : k * TB + (mb + 1) * fd],
                            outwq[:, k * 500 : (k + 1) * 500],
                            start=(k == 0),
                            stop=(k == C - 1),
                        )
                    fo = fpool.tile([128, 500], BF16, tag="fo", name="fo")
                    nc.scalar.copy(fo[:fd], pf[:fd, 0:500])
                    nc.sync.dma_start(
                        out=d_logits[
                            mb * fd : (mb + 1) * fd, nb * 500 : (nb + 1) * 500
                        ],
                        in_=fo[:fd],
                    )

    return nc


_CACHE = {}
TRACE = False
LAST_RESULTS = None


def _get_program(nS):
    if nS not in _CACHE:
        nc = _build(nS)
        nc.finalize()
        _CACHE[nS] = nc
    return _CACHE[nS]


def kernel(**inputs):
    input_ids = np.asarray(inputs["input_ids"])
    emb = np.asarray(inputs["emb"], np.float32)
    enc_W = np.asarray(inputs["enc_W"], np.float32)
    enc_b = np.asarray(inputs["enc_b"], np.float32)
    gen_W = np.asarray(inputs["gen_W"], np.float32)
    gen_b = np.asarray(inputs["gen_b"], np.float32)
    inf_W = np.asarray(inputs["inf_W"], np.float32)
    inf_b = np.asarray(inputs["inf_b"], np.float32)
    ns_g = np.asarray(inputs["ns_g"], np.float32)
    ns_b = np.asarray(inputs["ns_b"], np.float32)
    ne_g = np.asarray(inputs["ne_g"], np.float32)
    ne_b = np.asarray(inputs["ne_b"], np.float32)
    out_W = np.asarray(inputs["out_W"], np.float32)
    out_b = np.asarray(inputs["out_b"], np.float32)

    nB, nS = input_ids.shape
    assert (nB, nS) == (B, S), (nB, nS)

    tok = emb[input_ids]  # (B, S, D) host gather

    # host-side constant folding
    ctiles = np.zeros((15, 128, W64), np.float32)
    genw_t = np.zeros((L, 128, C * D), ml_dtypes.bfloat16)
    infw_t = np.zeros((L, 128, C * D), ml_dtypes.bfloat16)
    for j in range(L):
        ctiles[j] = _vec_T(gen_b[j])
        B2 = ne_b[j] @ inf_W[j].T + inf_b[j]
        G2 = ne_g[j] @ inf_W[j].T
        ctiles[3 + j] = _vec_T(B2)
        ctiles[6 + j] = _vec_T(G2)
        ctiles[9 + j] = _vec_T(ns_g[j])
        ctiles[12 + j] = _vec_T(ns_b[j])
        genw_t[j] = _wT_kmajor(gen_W[j], dt=ml_dtypes.bfloat16)
        infw_t[j] = _wT_kmajor(inf_W[j] * ne_g[j][None, :], dt=ml_dtypes.bfloat16)
    ctiles_packed = np.ascontiguousarray(
        ctiles.transpose(1, 0, 2).reshape(128, 15 * W64)
    ).astype(ml_dtypes.bfloat16)

    shared = {
        "tok_t": _tok_T(tok).astype(ml_dtypes.bfloat16),
        "encw_t": _wT_kmajor(enc_W, dt=ml_dtypes.bfloat16),
        "genw_t": genw_t,
        "infw_t": infw_t,
        "ctiles": ctiles_packed,
        "encb_c": np.ascontiguousarray(enc_b.reshape(C, 128).T).astype(np.float32),
        "ones_in": np.ones((128, 128), ml_dtypes.bfloat16),
    }

    nc = _get_program(nS)
    in_maps = []
    for i in range(NCORES):
        m = dict(shared)
        shard = out_W[i * VS : (i + 1) * VS]
        m["outw_t"] = np.concatenate(
            [
                _wT_kmajor(shard[q * 500 : (q + 1) * 500], dt=ml_dtypes.bfloat16)
                for q in range(8)
            ],
            axis=1,
        )
        in_maps.append(m)

    global LAST_RESULTS
    if TRACE:
        res = run_bass_kernel_spmd(nc, in_maps, list(range(NCORES)), trace=True)
    else:
        res = run_bass_kernel_spmd(nc, in_maps, list(range(NCORES)))
    LAST_RESULTS = res
    shards = []
    for i in range(NCORES):
        lg = (
            res.results[i]["logits"].astype(np.float32)
            .reshape(nS, B, VS).transpose(1, 0, 2)
        )
        shards.append(lg)
    logits = np.concatenate(shards, axis=2)  # (B, S, V)
    logits = logits + out_b[None, None, :]
    return logits.astype(np.float32)


if __name__ == "__main__":
    pass
